# revision 1
# baseline (speedup 1.0000x reference)
"""GCN classifier (2x GCNConv + JK-cat + mean-pool + linear) on 8 trn2 NeuronCores.

Strategy (per sharding hint): partition edges by destination node range; each
core owns the scatter-add for its 6250-node dst shard. Per layer: every core
computes h = x_shard @ W for its own nodes, AllGathers the full [N, 64] h
table (in two halves so gathers overlap the second collective), then
aggregates its edges with norm weighting:

  - non-self edges are sorted by local dst, split into A/B table halves
    (int16 gather index limit), and packed into 128-edge chunks
  - dma_gather (SWDGE MoE-style gather, 4 parallel queues) pulls h[src] rows
    (256B) for each chunk batch into SBUF
  - per chunk, one PE matmul computes psum[:, c0:c1] += msg^T @ S where
    S[e, j] = norm(e) at the edge's dst column: gather + scale + segment-sum
    fused on the tensor engine; psum blocks are pre-cleared by a bf16
    zero-matmul so chunk matmuls accumulate in any order
  - self loops never leave the core: their contribution dinv(n)^2 * h(n) is
    added per 128-node window by an ACT scaled copy + PE transpose into the
    same psum block (no gather descriptors)
  - ACT flushes psum -> relu(. + b) into the transposed activation table x1T

Pooling: PE-transpose x1T/x2T tiles to node-major xc, matmul against a
host-built S_pool (batch one-hot / graph size), AllReduce the [128, 64]
per-graph partials, final [64, 7] linear on-chip. All floating point math on
the data path runs on device in fp32; the host only prepares integer edge
orderings and the edge-norm constants (degree-derived, data-independent).
"""
import numpy as np

import concourse.bacc as bacc
import concourse.bass as bass
import concourse.mybir as mybir
import concourse.tile as tile
from concourse.bass_utils import run_bass_kernel_spmd

F32 = mybir.dt.float32
BF16 = mybir.dt.bfloat16
I16 = mybir.dt.int16

N, E, G = 50000, 800000, 64
D_IN, D_H, D_OUT = 128, 64, 7
NC = 8
SH = N // NC            # 6250 nodes per core
TILES = 49              # ceil(SH / 128)
SHP = TILES * 128       # 6272 padded shard rows
SHALF = SHP // 2        # 3136 rows per half-shard (A/B gather tables)
HROWS = SHALF * NC      # 25088 rows per half table (int16-addressable)
WIN = TILES             # 49 windows of 128 dst nodes
BLK_WINS = 4            # windows per psum block ([64, <=512])
NBLK = (WIN + BLK_WINS - 1) // BLK_WINS  # 13
NQ = 4                  # SWDGE queues

_cache = {}


def _schedule(src, dst, norm, batch_np):
    """Host preprocessing: per-core edge chunks + shared static schedule."""
    keep = src != dst  # self loops handled by the on-core diagonal path
    src, dst, norm = src[keep], dst[keep], norm[keep]
    core = dst // SH
    ldst = dst - core * SH
    srcc = src // SH
    srcr = src - srcc * SH
    stream = (srcr >= SHALF).astype(np.int64)
    idx16 = (srcc * SHALF + srcr - stream * SHALF).astype(np.int16)
    win = ldst // 128

    per = [[[None, None] for _ in range(WIN)] for _ in range(NC)]
    nch = np.zeros((WIN, 2), np.int64)
    for c in range(NC):
        m = core == c
        l, s, i16, nm, w = ldst[m], stream[m], idx16[m], norm[m], win[m]
        order = np.lexsort((l, s, w))
        l, s, i16, nm, w = l[order], s[order], i16[order], nm[order], w[order]
        for wi in range(WIN):
            wm = w == wi
            for t in range(2):
                tm = wm & (s == t)
                ne = int(tm.sum())
                per[c][wi][t] = (l[tm], i16[tm], nm[tm])
                nch[wi, t] = max(nch[wi, t], (ne + 127) // 128)

    slots = []  # (win, t, k)
    units = []  # (blk, t, slot_lo, slot_hi)
    # A-stream leads B by 2 blocks so the in-order Pool engine never stalls
    # on the second AllGather half
    pairs = []
    for b in range(NBLK + 1):
        if b < NBLK:
            pairs.append((b, 0))
        if b >= 1:
            pairs.append((b - 1, 1))
    for b, t in pairs:
        wins = range(b * BLK_WINS, min((b + 1) * BLK_WINS, WIN))
        s0 = len(slots)
        for wi in wins:
            for k in range(nch[wi, t]):
                slots.append((wi, t, k))
        if len(slots) > s0:
            units.append((b, t, s0, len(slots)))
    nslot = len(slots)

    c0s = np.full(nslot, 1 << 30, np.int64)
    c1s = np.full(nslot, -1, np.int64)
    slot_pos = {}
    for si, (wi, t, k) in enumerate(slots):
        slot_pos[(wi, t, k)] = si
    for c in range(NC):
        for wi in range(WIN):
            for t in range(2):
                l, i16, nm = per[c][wi][t]
                for k in range((len(l) + 127) // 128):
                    si = slot_pos[(wi, t, k)]
                    seg = l[k * 128:(k + 1) * 128] - wi * 128
                    c0s[si] = min(c0s[si], seg.min())
                    c1s[si] = max(c1s[si], seg.max() + 1)
    c0s = np.where(c1s < 0, 0, c0s)
    c1s = np.maximum(c1s, c0s + 1)
    ms = c1s - c0s
    s_off = np.zeros(nslot + 1, np.int64)
    s_off[1:] = np.cumsum(ms)
    stot = int(s_off[-1])

    idx_flat = np.zeros((NC, nslot * 128), np.int16)
    s_tab = np.zeros((NC, 128, stot), np.float32)
    for c in range(NC):
        for wi in range(WIN):
            for t in range(2):
                l, i16, nm = per[c][wi][t]
                for k in range((len(l) + 127) // 128):
                    si = slot_pos[(wi, t, k)]
                    sl = slice(k * 128, min((k + 1) * 128, len(l)))
                    n_in = sl.stop - sl.start
                    idx_flat[c, si * 128: si * 128 + n_in] = i16[sl]
                    rel = l[sl] - wi * 128 - c0s[si]
                    s_tab[c, np.arange(n_in), s_off[si] + rel] = nm[sl]

    idx_w = np.zeros((NC, 128, nslot * 8), np.int16)
    for c in range(NC):
        w = idx_flat[c].reshape(-1, 16).T
        idx_w[c] = np.tile(w, (8, 1))

    col0 = np.array(
        [(wi - (wi // BLK_WINS) * BLK_WINS) * 128 + c0s[si]
         for si, (wi, t, k) in enumerate(slots)], np.int64)

    cnt = np.maximum(np.bincount(batch_np, minlength=G), 1).astype(np.float32)
    s_pool = np.zeros((NC, 128, TILES, G), np.float32)
    for c in range(NC):
        r = np.arange(SH)
        g = batch_np[c * SH:(c + 1) * SH]
        s_pool[c, r % 128, r // 128, g] = 1.0 / cnt[g]

    meta = dict(slots=slots, units=units, ms=ms, s_off=s_off, stot=stot,
                col0=col0, nslot=nslot)
    return meta, idx_w, s_tab, s_pool


def _build(meta):
    nslot, stot = meta["nslot"], meta["stot"]
    units, ms, s_off, col0 = meta["units"], meta["ms"], meta["s_off"], meta["col0"]

    nc = bacc.Bacc("TRN2", target_bir_lowering=False, debug=False,
                   num_devices=NC, num_swdge_queues=NQ)

    xT_d = nc.dram_tensor("xT", [D_IN, SHP], F32, kind="ExternalInput")
    idx_d = nc.dram_tensor("idx", [128, nslot * 8], I16, kind="ExternalInput")
    s_d = nc.dram_tensor("stab", [128, stot], F32, kind="ExternalInput")
    spool_d = nc.dram_tensor("spool", [128, TILES, G], F32, kind="ExternalInput")
    w1_d = nc.dram_tensor("W1", [D_IN, D_H], F32, kind="ExternalInput")
    w2_d = nc.dram_tensor("W2", [D_H, D_H], F32, kind="ExternalInput")
    wl_d = nc.dram_tensor("Wlin", [2 * D_H, D_OUT], F32, kind="ExternalInput")
    b1_d = nc.dram_tensor("b1", [D_H, 1], F32, kind="ExternalInput")
    b2_d = nc.dram_tensor("b2", [D_H, 1], F32, kind="ExternalInput")
    bl_d = nc.dram_tensor("blin_t", [G, D_OUT], F32, kind="ExternalInput")
    eye_d = nc.dram_tensor("eye64", [D_H, D_H], F32, kind="ExternalInput")
    eye128_d = nc.dram_tensor("eye128", [128, 128], F32, kind="ExternalInput")
    dinv2_d = nc.dram_tensor("dinv2", [128, TILES], F32, kind="ExternalInput")
    out_d = nc.dram_tensor("out", [G, D_OUT], F32, kind="ExternalOutput")

    h_loc = [[nc.dram_tensor(f"h{i}_loc{ab}", [SHALF, D_H], F32)
              for ab in "AB"] for i in (1, 2)]
    h_full = [[nc.dram_tensor(f"h{i}_full{ab}", [HROWS, D_H], F32,
                              addr_space="Shared") for ab in "AB"] for i in (1, 2)]
    pool_loc = nc.dram_tensor("pool_loc", [128, G], F32)
    pool_full = nc.dram_tensor("pool_full", [128, G], F32, addr_space="Shared")

    max_u = max(u[3] - u[2] for u in units)
    max_s = max(int(s_off[u[3]] - s_off[u[2]]) for u in units)
    blk_w = [min((b + 1) * BLK_WINS, WIN) * 128 - b * BLK_WINS * 128
             for b in range(NBLK)]

    with tile.TileContext(nc) as tc:
        with (
            tc.tile_pool(name="persist", bufs=1) as pp,
            tc.tile_pool(name="psA", bufs=3, space="PSUM") as psA,
        ):
            w1_t = pp.tile([D_IN, D_H], F32)
            w2_t = pp.tile([D_H, D_H], F32)
            wl_t = pp.tile([2 * D_H, D_OUT], F32)
            b_t = [pp.tile([D_H, 1], F32, name=f"b{i}", tag=f"b{i}") for i in range(2)]
            bl_t = pp.tile([G, D_OUT], F32)
            eye_t = pp.tile([D_H, D_H], F32)
            eye128_t = pp.tile([128, 128], F32)
            dinv2_t = pp.tile([128, TILES], F32)
            zz_t = pp.tile([128, 512], BF16)
            idx_t = pp.tile([128, nslot * 8], I16)
            spool_t = pp.tile([128, TILES, G], F32)
            xT_t = pp.tile([D_IN, SHP], F32)
            xaT = [pp.tile([D_H, SHP], F32, name=f"xaT{i}", tag=f"xaT{i}")
                   for i in range(2)]
            xc_t = pp.tile([128, TILES, 2 * D_H], F32)
            stage = pp.tile([128, TILES, D_H], F32)

            nc.sync.dma_start(w1_t[:], w1_d[:])
            nc.sync.dma_start(w2_t[:], w2_d[:])
            nc.sync.dma_start(wl_t[:], wl_d[:])
            nc.sync.dma_start(b_t[0][:], b1_d[:])
            nc.sync.dma_start(b_t[1][:], b2_d[:])
            nc.sync.dma_start(bl_t[:], bl_d[:])
            nc.sync.dma_start(eye_t[:], eye_d[:])
            nc.sync.dma_start(eye128_t[:], eye128_d[:])
            nc.sync.dma_start(dinv2_t[:], dinv2_d[:])
            idx_cols = nslot * 8
            iq = (idx_cols + 3) // 4
            for q in range(4):
                i0, i1 = q * iq, min((q + 1) * iq, idx_cols)
                if i1 > i0:
                    nc.scalar.dma_start(idx_t[:, i0:i1], idx_d[:, i0:i1])
            nc.scalar.dma_start(spool_t[:], spool_d[:])
            for q in range(4):
                nc.sync.dma_start(
                    xT_t[:, q * 1568:(q + 1) * 1568],
                    xT_d[:, q * 1568:(q + 1) * 1568])
            nc.vector.memset(zz_t[:], 0.0)

            def phase_a_tile(layer, t):
                ps = psA.tile([128, D_H], F32, name="psa", tag="psa", bufs=2)
                if layer == 0:
                    lhsT = xT_t[:, t * 128:(t + 1) * 128]
                    wt = w1_t
                else:
                    lhsT = xaT[0][:, t * 128:(t + 1) * 128]
                    wt = w2_t
                nc.tensor.matmul(ps[:], lhsT, wt[:])
                nc.scalar.copy(stage[:, t, :], ps[:])

            def emit_ag(layer, half):
                la, lb = h_loc[layer]
                fa, fb = h_full[layer]
                if half == 0:
                    # rows 0..3071 = tiles 0..23; 3072..3135 = tile 24 p<64
                    nc.sync.dma_start(
                        la[0:3072, :].rearrange("(t p) f -> p t f", p=128),
                        stage[:, 0:24, :])
                    nc.sync.dma_start(la[3072:3136, :], stage[0:64, 24, :])
                    nc.gpsimd.collective_compute(
                        "AllGather", mybir.AluOpType.bypass,
                        replica_groups=[list(range(NC))],
                        ins=[la[:]], outs=[fa[:]])
                else:
                    nc.sync.dma_start(lb[0:64, :], stage[64:128, 24, :])
                    nc.sync.dma_start(
                        lb[64:3136, :].rearrange("(t p) f -> p t f", p=128),
                        stage[:, 25:49, :])
                    nc.gpsimd.collective_compute(
                        "AllGather", mybir.AluOpType.bypass,
                        replica_groups=[list(range(NC))],
                        ins=[lb[:]], outs=[fb[:]])

            def phase_a(layer):
                for t in range(TILES):
                    phase_a_tile(layer, t)
                emit_ag(layer, 0)
                emit_ag(layer, 1)

            with (
                tc.tile_pool(name="msg", bufs=4) as mpool,
                tc.tile_pool(name="stabp", bufs=4) as spool_p,
                tc.tile_pool(name="selfp", bufs=3) as selfp,
                tc.tile_pool(name="psB", bufs=4, space="PSUM") as psB,
            ):
                def open_block(b):
                    """zero psum block + add self-loop diagonal from stage."""
                    ps = psB.tile([D_H, 512], F32, name="psb", tag="psb")
                    nc.tensor.matmul(
                        ps[:, 0:blk_w[b]], zz_t[:, 0:D_H],
                        zz_t[:, 0:blk_w[b]], start=True, stop=True)
                    for wi in range(b * BLK_WINS, min((b + 1) * BLK_WINS, WIN)):
                        sm = selfp.tile([128, D_H], F32, name="selfm", tag="selfm")
                        nc.scalar.activation(
                            sm[:], stage[:, wi, :],
                            mybir.ActivationFunctionType.Copy,
                            scale=dinv2_t[:, wi:wi + 1])
                        c0 = (wi - b * BLK_WINS) * 128
                        nc.tensor.matmul(
                            ps[:, c0:c0 + 128], sm[:], eye128_t[:],
                            start=False, stop=True, skip_group_check=True)
                    return ps

                def layer_main(layer, on_block_done=None):
                    ps_blk = [None] * NBLK
                    done_units = [0] * NBLK
                    units_per_blk = [0] * NBLK
                    for (b, t, a0, a1) in units:
                        units_per_blk[b] += 1
                    qn = 0
                    for ui, (b, t, a0, a1) in enumerate(units):
                        nchu = a1 - a0
                        half = (nchu + 1) // 2
                        maxh = (max_u + 1) // 2
                        parts = []
                        for h0, h1 in ((a0, a0 + half), (a0 + half, a1)):
                            nh = h1 - h0
                            if nh <= 0:
                                continue
                            mt = mpool.tile([128, maxh, D_H], F32,
                                            name="msg", tag="msg", bufs=6)
                            nc.gpsimd.dma_gather(
                                mt[:, 0:nh, :], h_full[layer][t][:],
                                idx_t[:, h0 * 8:h1 * 8],
                                nh * 128, nh * 128, D_H,
                                single_packet=False, queue_num=qn % NQ)
                            qn += 1
                            parts.append((h0, h1, mt))
                        st_t = spool_p.tile([128, max_s], F32,
                                            name="stab", tag="stab")
                        u_soff = int(s_off[a0])
                        u_slen = int(s_off[a1] - u_soff)
                        nc.sync.dma_start(st_t[:, 0:u_slen],
                                          s_d[:, u_soff:u_soff + u_slen])
                        if ps_blk[b] is None:
                            ps_blk[b] = open_block(b)
                        for (h0, h1, mt) in parts:
                            for si in range(h0, h1):
                                m = int(ms[si])
                                so = int(s_off[si] - u_soff)
                                c0 = int(col0[si])
                                nc.tensor.matmul(
                                    ps_blk[b][:, c0:c0 + m],
                                    mt[:, si - h0, :],
                                    st_t[:, so:so + m],
                                    start=False, stop=True,
                                    skip_group_check=True)
                        done_units[b] += 1
                        if done_units[b] == units_per_blk[b]:
                            w0 = b * BLK_WINS * 128
                            nc.scalar.activation(
                                xaT[layer][:, w0:w0 + blk_w[b]],
                                ps_blk[b][:, 0:blk_w[b]],
                                mybir.ActivationFunctionType.Relu,
                                bias=b_t[layer][:])
                            if on_block_done is not None:
                                on_block_done(b)

                def transpose_tile(half, t):
                    ps = psA.tile([128, D_H], F32, name="pst", tag="pst",
                                  bufs=1)
                    nc.tensor.transpose(
                        ps[:], xaT[half][:, t * 128:(t + 1) * 128], eye_t[:])
                    nc.scalar.copy(
                        xc_t[:, t, half * D_H:(half + 1) * D_H], ps[:])

                pool_state = {}

                phase_a(0)
                layer_main(0)
                for t in range(TILES):
                    transpose_tile(0, t)
                phase_a(1)
                layer_main(1)
                for t in range(TILES):
                    transpose_tile(1, t)
                pool_state["ps"] = psA.tile(
                    [128, G], F32, name="poolps", tag="poolps", bufs=1)
                for t in range(TILES):
                    nc.tensor.matmul(
                        pool_state["ps"][:], xc_t[:, t, :], spool_t[:, t, :],
                        start=(t == 0), stop=(t == TILES - 1),
                        skip_group_check=True)

                pool_ps = pool_state["ps"]
                pool_sb = pp.tile([128, G], F32)
                nc.scalar.copy(pool_sb[:], pool_ps[:])
                nc.sync.dma_start(pool_loc[:], pool_sb[:])
                nc.gpsimd.collective_compute(
                    "AllReduce", mybir.AluOpType.add,
                    replica_groups=[list(range(NC))],
                    ins=[pool_loc[:]], outs=[pool_full[:]])
                pooled_t = pp.tile([128, G], F32)
                nc.sync.dma_start(pooled_t[:], pool_full[:])
                fin_ps = psA.tile([G, D_OUT], F32, name="fin", tag="pst",
                                  bufs=1)
                nc.tensor.matmul(fin_ps[:], pooled_t[:], wl_t[:])
                out_t = pp.tile([G, D_OUT], F32)
                nc.vector.tensor_add(out_t[:], fin_ps[:], bl_t[:])
                nc.sync.dma_start(out_d[:], out_t[:])

    nc.compile()
    return nc


def _prep_inputs(x, edge_index, batch, W1, b1, W2, b2, Wlin, blin):
    src = np.concatenate([np.asarray(edge_index[0]), np.arange(N)]).astype(np.int64)
    dst = np.concatenate([np.asarray(edge_index[1]), np.arange(N)]).astype(np.int64)
    deg = np.bincount(dst, minlength=N).astype(np.float64)
    dinv = 1.0 / np.sqrt(np.maximum(deg, 1e-12))
    norm = (dinv[src] * dinv[dst]).astype(np.float32)
    batch_np = np.asarray(batch).astype(np.int64)

    meta, idx_w, s_tab, s_pool = _schedule(src, dst, norm, batch_np)

    x = np.asarray(x, np.float32)
    xT = np.zeros((NC, D_IN, SHP), np.float32)
    for c in range(NC):
        xT[c, :, 0:SH] = x[c * SH:(c + 1) * SH].T

    # diagonal weight: (# self edges incl. added loop) * dinv^2
    mult = np.bincount(dst[src == dst], minlength=N).astype(np.float64)
    dinv2 = np.zeros((NC, 128, TILES), np.float32)
    for c in range(NC):
        r = np.arange(SH)
        dinv2[c, r % 128, r // 128] = (mult[c * SH + r] *
                                       dinv[c * SH + r] ** 2).astype(np.float32)

    com = dict(
        W1=np.asarray(W1, np.float32), W2=np.asarray(W2, np.float32),
        Wlin=np.asarray(Wlin, np.float32),
        b1=np.asarray(b1, np.float32).reshape(D_H, 1),
        b2=np.asarray(b2, np.float32).reshape(D_H, 1),
        blin_t=np.tile(np.asarray(blin, np.float32), (G, 1)),
        eye64=np.eye(D_H, dtype=np.float32),
        eye128=np.eye(128, dtype=np.float32),
    )
    in_maps = [
        dict(com, xT=xT[c], idx=idx_w[c], stab=s_tab[c], spool=s_pool[c],
             dinv2=dinv2[c])
        for c in range(NC)
    ]
    return meta, in_maps


def kernel(x, edge_index, batch, W1, b1, W2, b2, Wlin, blin, _trace=False):
    meta, in_maps = _prep_inputs(x, edge_index, batch, W1, b1, W2, b2, Wlin, blin)
    key = (meta["nslot"], meta["stot"], tuple(meta["ms"].tolist()))
    if key not in _cache:
        _cache.clear()
        _cache[key] = _build(meta)
    nc = _cache[key]
    res = run_bass_kernel_spmd(nc, in_maps, list(range(NC)), trace=_trace)
    out = res.results[0]["out"].astype(np.float32)
    if _trace:
        return out, res.exec_time_ns
    return out

